# revision 24
# baseline (speedup 1.0000x reference)
"""Trainium2 Bass kernel for topk_masking:  out = X + alpha * (top32_mask(A) @ X).

Row-parallel across 8 NeuronCores (A sharded [1024, 8192] per core, X
replicated).  Per 128-row batch on each core:
  * VectorE: per-segment max8 -> candidate top-8s, then 4 rounds of
    max+match_replace -> top-32 values; t32 = 32nd largest.  Exact unless >8
    of a row's top-32 fall in one segment (detected and host-fixed).
  * ScalarE: maskpm = Sign(A - prevfloat(t32)) in fp8e4 (+1 selected, -1 not),
    with fused accumulation as an exactness detector.
  * GPSIMD dma_gather(transpose): fp8 mask pairs viewed as int16 are
    transposed per super-batch of 512 rows (the gather transposes at 16-bit
    granularity), so partition p holds columns (2p, 2p+1) byte-interleaved.
  * TensorE (operands swapped for fast weight loads): X chunk-halves (bf16,
    contiguous, from a host-prepared parity-interleaved layout) are the
    stationary operand; the strided fp8 maskT is the moving operand at N=512.
    psumT[d, r] accumulates (maskpm @ X).T over all 64 contraction chunks.
  * ScalarE: outT = (a/2) * psumT in fp16 (activation Copy with per-partition
    scale); DMA out on the ACT queue so the sync queue streams only A.
Host: out = outT.T + X + (a/2) * colsum(bf16(X)); the bf16 colsum matches the
device's bf16 X so unselected rows' rounding errors cancel (mask01 @ X =
(maskpm @ X + colsum X)/2).  Rows whose detector count != 32 are recomputed
exactly on host (~11 rows for this data).
"""

import os
import numpy as np

N = 8192
D = 256
K = 32
NCORES = 8
RPC = N // NCORES          # rows per core = 1024
BATCH = 128
NBATCH = RPC // BATCH      # 8
SUPER = 512                # rows per matmul super-batch
NSUPER = RPC // SUPER      # 2
BPS = SUPER // BATCH       # 4 batches per super
SEG = int(os.environ.get("TOPK_SEG", "512"))
NCH = N // 128             # 64 contraction chunks (of 128)
NPCH = N // 256            # 32 pair-chunks (of 256)
NEG_BIG = -1e30
ONE_MINUS_EPS = float(np.float32(1.0) - np.float32(2.0 ** -24))

last_results = None
_nc_cache = {}


def _build_cached(loop_reps=1, seg=None):
    key = (loop_reps, seg or SEG)
    if key not in _nc_cache:
        _nc_cache[key] = _build(loop_reps, seg)
    return _nc_cache[key]


def _build(loop_reps=1, seg=None):
    import concourse.bacc as bacc
    import concourse.mybir as mybir
    from concourse.tile import TileContext
    from concourse import library_config

    seg = seg or SEG
    nseg = N // seg            # segments per full row
    fp32 = mybir.dt.float32
    bf16 = mybir.dt.bfloat16
    fp16 = mybir.dt.float16
    fp8 = mybir.dt.float8e4
    i16 = mybir.dt.int16
    Sign = mybir.ActivationFunctionType.Sign
    Copy = mybir.ActivationFunctionType.Copy

    QW = N // 4                # quarter width (2048)

    nc = bacc.Bacc("TRN2", debug=False)
    a_in = nc.declare_dram_parameter("a", [RPC, N], fp32, isOutput=False)
    xp_in = nc.declare_dram_parameter("xpar", [128, NCH * D], bf16, isOutput=False)
    al_in = nc.declare_dram_parameter("alpha_h", [128, 1], fp32, isOutput=False)
    ti_in = nc.declare_dram_parameter("tidx", [128, SUPER // 16], mybir.dt.int16,
                                      isOutput=False)
    outT_ext = nc.declare_dram_parameter("outT", [D, RPC], fp16, isOutput=True)
    cnt_ext = nc.declare_dram_parameter("count", [RPC, 1], fp32, isOutput=True)

    abufs = int(os.environ.get("TOPK_ABUFS", "3"))
    mbbufs = int(os.environ.get("TOPK_MBBUFS", "5"))
    mtbufs = int(os.environ.get("TOPK_MTBUFS", "3"))

    with TileContext(nc) as tc:
        with (
            tc.tile_pool(name="persist", bufs=1) as persist,
            tc.tile_pool(name="apool", bufs=abufs) as apool,
            tc.tile_pool(name="mpool", bufs=mbbufs) as mpool,
            tc.tile_pool(name="mtpool", bufs=mtbufs) as mtpool,
            tc.tile_pool(name="small", bufs=2) as small,
            tc.tile_pool(name="accp", bufs=3) as accp,
            tc.tile_pool(name="otp", bufs=3) as otp,
            tc.tile_pool(name="psum", bufs=2, space="PSUM") as psum_pool,
        ):
            nc.gpsimd.load_library(library_config.mlp)

            at_tiles = {}
            QW4 = N // 4

            def load_at(b):
                qs = []
                for q in range(4):
                    atq = apool.tile([128, QW4], fp32, tag=f"at{q}")
                    nc.sync.dma_start(
                        out=atq[:],
                        in_=a_in[b * BATCH:(b + 1) * BATCH, q * QW4:(q + 1) * QW4])
                    qs.append(atq)
                at_tiles[b] = qs

            if loop_reps == 1:
                load_at(0)
                load_at(1)

            tidx = persist.tile([128, SUPER // 16], mybir.dt.int16)
            nc.scalar.dma_start(out=tidx[:], in_=ti_in[:])

            # X resident in bf16, parity-interleaved chunk-major:
            # xpar[p, g*D + d] = X[(g>>1)*256 + 2p + (g&1), d]
            xpar = persist.tile([128, NCH * D], bf16)
            nc.scalar.dma_start(out=xpar[:], in_=xp_in[:])
            alpha_h = persist.tile([128, 1], fp32)
            nc.scalar.dma_start(out=alpha_h[:], in_=al_in[:])
            cnt_all = persist.tile([128, NBATCH], fp32)

            xv = xpar[:].rearrange("p (g d) -> p g d", d=D)
            acc_tiles = {}

            def scan_batch(b):
                if b + 2 < NBATCH:
                    load_at(b + 2)
                atq = at_tiles.pop(b)

                # per-segment top-8 candidates, quarter by quarter
                qseg = QW4 // seg
                cands = small.tile([128, nseg * 8], fp32)
                for q in range(4):
                    for s in range(qseg):
                        g = q * qseg + s
                        nc.vector.max(out=cands[:, g * 8:(g + 1) * 8],
                                      in_=atq[q][:, s * seg:(s + 1) * seg])

                # top-32 of candidates -> t32
                v8 = small.tile([128, K], fp32)
                for r in range(4):
                    nc.vector.max(out=v8[:, r * 8:(r + 1) * 8], in_=cands[:])
                    if r < 3:
                        nc.vector.match_replace(
                            out=cands[:], in_to_replace=v8[:, r * 8:(r + 1) * 8],
                            in_values=cands[:], imm_value=NEG_BIG)

                # detector readout for batch b-1 (acc ready; no DVE stall)
                if b - 1 in acc_tiles:
                    accp_t = acc_tiles.pop(b - 1)
                    nc.vector.reduce_sum(cnt_all[:, b - 1:b], accp_t[:],
                                         axis=mybir.AxisListType.X)

                # neg_tprime = -prevfloat(t32) = t32 * -(1 - 2^-24)   (on ACT)
                ntp = small.tile([128, 1], fp32)
                nc.scalar.activation(out=ntp[:], in_=v8[:, K - 1:K], func=Copy,
                                     scale=-ONE_MINUS_EPS)
                return atq, ntp

            def body():
                for S in range(NSUPER):
                    # 4 maskb tiles for this super (one per quarter), each
                    # [128, 4 batches * 2048] fp8, written by 4 Sign slices
                    mbs = [mpool.tile([128, BPS * QW], fp8, tag="mb",
                                      name=f"mb{S}_{q}")
                           for q in range(4)]
                    for j in range(BPS):
                        b = S * BPS + j
                        atq, ntp = scan_batch(b)
                        acc4 = accp.tile([128, 4], fp32)
                        acc_tiles[b] = acc4
                        for q in range(4):
                            nc.scalar.activation(
                                out=mbs[q][:, j * QW:(j + 1) * QW],
                                in_=atq[q][:], func=Sign,
                                bias=ntp[:, 0:1], scale=1.0,
                                accum_out=acc4[:, q:q + 1])

                    # psumT halves: psT[d, r] accumulates (maskpm @ X).T
                    pss = [psum_pool.tile([128, SUPER], fp32, tag=f"ps{h}",
                                          name=f"ps{S}_{h}")
                           for h in range(2)]

                    for q in range(4):
                        maskT = mtpool.tile([128, (QW // 256) * SUPER], i16)
                        nc.gpsimd.dma_gather(
                            out_ap=maskT[:].rearrange("p (c i) -> p c i", i=SUPER),
                            in_ap=mbs[q][:].bitcast(i16), idxs_ap=tidx[:],
                            num_idxs=SUPER, num_idxs_reg=SUPER,
                            elem_size=QW // 2, transpose=True,
                            sbuf_tokens_per_rank=128,
                            sbuf_free_dim_per_rank=QW)
                        mT = maskT[:].bitcast(fp8).rearrange(
                            "p (c i par) -> p c i par", i=SUPER, par=2)

                        for c in range(QW // 256):
                            for par in range(2):
                                g = q * 16 + c * 2 + par
                                for h in range(2):
                                    nc.tensor.matmul(
                                        pss[h][:],
                                        lhsT=xv[:, g, h * 128:(h + 1) * 128],
                                        rhs=mT[:, c, :, par],
                                        start=(g == 0),
                                        stop=(g == NCH - 1))

                    # outT = (alpha/2) * psumT   (ACT copy; out-DMA on ACT q)
                    for h in range(2):
                        otT = otp.tile([128, SUPER], fp16)
                        nc.scalar.activation(out=otT[:], in_=pss[h][:],
                                             func=Copy, scale=alpha_h[:, 0:1])
                        nc.scalar.dma_start(
                            out=outT_ext[h * 128:(h + 1) * 128,
                                         S * SUPER:(S + 1) * SUPER],
                            in_=otT[:])

                # last batch's detector readout
                b = NSUPER * BPS - 1
                accp_t = acc_tiles.pop(b)
                nc.vector.reduce_sum(cnt_all[:, b:b + 1], accp_t[:],
                                     axis=mybir.AxisListType.X)

            if loop_reps == 1:
                body()
            else:
                with tc.For_i(0, loop_reps, 1):
                    load_at(0)
                    load_at(1)
                    body()

            # counts: cnt_all[p, b] -> count[b*128 + p]
            nc.sync.dma_start(
                out=cnt_ext.rearrange("(b p) one -> p (b one)", p=128),
                in_=cnt_all[:],
            )
    nc.compile()
    return nc


def _tidx():
    t = np.zeros((16, SUPER // 16), np.int16)
    for i in range(SUPER):
        t[i % 16, i // 16] = i
    return np.tile(t, (8, 1))


def _xpar(X):
    import ml_dtypes
    # xpar[p, g, d] = X[(g>>1)*256 + 2p + (g&1), d]
    Xr = X.reshape(NPCH, 128, 2, D)          # [cc, p, parity, d]
    xp = Xr.transpose(1, 0, 2, 3).reshape(128, NCH * D)
    return np.ascontiguousarray(xp).astype(ml_dtypes.bfloat16)


def make_in_maps(A, X, alpha):
    xpar = _xpar(X)
    alpha_h = np.full((128, 1), np.float32(alpha) / np.float32(2.0), np.float32)
    tidx = _tidx()
    return [{
        "a": A[c * RPC:(c + 1) * RPC],
        "xpar": xpar,
        "alpha_h": alpha_h,
        "tidx": tidx,
    } for c in range(NCORES)]


def kernel(**inputs):
    global last_results
    from concourse.bass_utils import run_bass_kernel_spmd

    A = np.ascontiguousarray(np.asarray(inputs["A"], dtype=np.float32))
    X = np.ascontiguousarray(np.asarray(inputs["X"], dtype=np.float32))
    alpha = np.float32(np.asarray(inputs["alpha"]))
    k = int(np.asarray(inputs["k"]))
    assert A.shape == (N, N) and X.shape == (N, D)
    if k != K or float(alpha) == 0.0:
        # Safety net for unexpected k / alpha: exact host computation.
        idx = np.argsort(-A, axis=1, kind="stable")[:, :k]
        agg = X[idx].sum(axis=1, dtype=np.float32)
        return (X + alpha * agg).astype(np.float32)

    nc = _build_cached()
    in_maps = make_in_maps(A, X, alpha)

    trace = bool(int(os.environ.get("TOPK_TRACE", "0")))
    res = run_bass_kernel_spmd(nc, in_maps, core_ids=list(range(NCORES)),
                               trace=trace)
    last_results = res

    # Device returns (alpha/2) * (maskpm @ X).T per core; the host adds
    # X + (alpha/2) * colsum(bf16(X)) -- matching the device's bf16 X so the
    # unselected rows' bf16 rounding errors cancel exactly.
    import ml_dtypes
    Xb = X.astype(ml_dtypes.bfloat16).astype(np.float32)
    cs_term = (np.float32(alpha) / np.float32(2.0)) * Xb.sum(
        axis=0, dtype=np.float32)
    out = np.concatenate(
        [r["outT"].astype(np.float32).T for r in res.results],
        axis=0) + (X + cs_term[None, :])
    accs = np.concatenate([r["count"] for r in res.results], axis=0)[:, 0]

    # Host fallback for rows where the device selection is not exactly top-k
    # (boundary value ties, segment overflow, Sign hitting exact zero).
    bad = np.flatnonzero(accs != np.float32(2 * K - N))
    for r in bad:
        order = np.argsort(-A[r], kind="stable")[:K]
        out[r] = X[r] + alpha * X[order].sum(axis=0, dtype=np.float32)

    return out.astype(np.float32, copy=False)
